# revision 3
# baseline (speedup 1.0000x reference)
"""VQ codebook encoding (nn_Encoding) kernel for 8 Trainium2 NeuronCores.

Reference computation (per batch b):
    xf = x[b].reshape(C, N).T                     # (N, C), N = H*W
    s_nk = scale_k * (||x_n||^2 - 2 x_n.c_k + ||c_k||^2)
    aw = softmax_k(s)
    enc[b] = aw^T xf - (sum_n aw)_k c_k           # (K, C)

Distribution: data-parallel over batch B across the 8 cores (2 batches per
core), codewords/scale replicated.

v2 design (per batch, per core):
  - host: xh = bf16(x) (the only big stream, 2B/elem); x2_n = ||x_n||^2
    computed exactly on host in f64 and streamed as a tiny (128, 72) f32
    tile; softmax offset m folded into bias (exact softmax is invariant
    to a per-pixel-constant offset).
  - mm1 (PE):  T_nk = sum_c xh * W1, W1 = -2*scale_k*c_k (bf16),
               a-tile stationary (shared with the transpose below)
  - transpose: xh tiles transposed on PE (transpose-mode, bf16 PSUM out),
               evacuated to SBUF by DVE/ACT alternating
  - z (DVE):   z = scale_k * x2_n + T + bias  (STT + add)
  - softmax:   e = exp(z) on ACT with accum_out = d, aw = e * (1/d) bf16
  - mm2 (PE):  enc[k, c] += sum_n aw * xT   -- aw STATIONARY (32-col load),
               xT moving (512 streams); awsum via ones column (1 stream).
               Output lands directly in (K, C) layout.
  - tail:      enc += awsum_k * (-c_kc) (one STT on 32 partitions), DMA out.
  - PE instruction stream is software-pipelined: tile i's mm2 is issued
    after tile i+PIPE's mm1/transpose so the PE never waits on the
    cross-engine softmax latency.
"""

import os

os.environ.setdefault("JAX_PLATFORMS", "")

import numpy as np
import ml_dtypes
from contextlib import ExitStack

import concourse.bacc as bacc
import concourse.bass as bass
import concourse.mybir as mybir
import concourse.tile as tile
from concourse.bass_utils import run_bass_kernel_spmd

bf16 = ml_dtypes.bfloat16
F32 = mybir.dt.float32
BF = mybir.dt.bfloat16

B, C, H, W = 16, 512, 96, 96
N = H * W            # 9216
K = 32
NCORES = 8
BPC = B // NCORES    # batches per core = 2
NCH = 8              # N chunks per batch
NC = N // NCH        # 1152 pixels per chunk
NT = NC // 128       # 9 tiles per chunk
CCH = C // 128       # 4 contraction chunks
NTILES = N // 128    # 72 tiles per batch
PIPE = 8             # mm2 issue lag (tiles) to hide softmax latency

_mult = mybir.AluOpType.mult
_add = mybir.AluOpType.add

_compiled = {}


def _build_program(reps=1):
    nc = bacc.Bacc("TRN2", target_bir_lowering=False, debug=False,
                   num_devices=NCORES)

    xh_d = nc.dram_tensor("xh", [BPC, CCH, 128, N], BF, kind="ExternalInput").ap()
    x2_d = nc.dram_tensor("x2p", [BPC, 128, NTILES], F32, kind="ExternalInput").ap()
    w1t_d = nc.dram_tensor("w1t", [128, CCH, K], BF, kind="ExternalInput").ap()
    scaleb_d = nc.dram_tensor("scaleb", [128, K], F32, kind="ExternalInput").ap()
    biasb_d = nc.dram_tensor("biasb", [128, K], F32, kind="ExternalInput").ap()
    cwneg_d = nc.dram_tensor("cwneg", [K, C], F32, kind="ExternalInput").ap()
    ident_d = nc.dram_tensor("ident", [128, 128], BF, kind="ExternalInput").ap()
    onescol_d = nc.dram_tensor("ones_col", [128, 1], BF, kind="ExternalInput").ap()
    out_d = nc.dram_tensor("enc", [BPC, K, C], F32, kind="ExternalOutput").ap()

    with tile.TileContext(nc) as tc, ExitStack() as ctx:
        const = ctx.enter_context(tc.tile_pool(name="const", bufs=1))
        xpool = ctx.enter_context(tc.tile_pool(name="xh", bufs=2))
        x2pool = ctx.enter_context(tc.tile_pool(name="x2", bufs=2))
        psT = ctx.enter_context(tc.tile_pool(name="psT", bufs=3, space="PSUM"))
        psX = ctx.enter_context(tc.tile_pool(name="psX", bufs=3, space="PSUM"))
        psE = ctx.enter_context(tc.tile_pool(name="psE", bufs=1, space="PSUM"))
        psA = ctx.enter_context(tc.tile_pool(name="psA", bufs=1, space="PSUM"))
        sbX = ctx.enter_context(tc.tile_pool(name="sbX", bufs=PIPE + 3))
        sbSmall = ctx.enter_context(tc.tile_pool(name="sbSmall", bufs=6))
        sbZ = ctx.enter_context(tc.tile_pool(name="sbZ", bufs=6))
        sbAw = ctx.enter_context(tc.tile_pool(name="sbAw", bufs=PIPE + 3))
        sbOut = ctx.enter_context(tc.tile_pool(name="sbOut", bufs=2))

        w1t = const.tile([128, CCH, K], BF)
        nc.sync.dma_start(w1t[:], w1t_d)
        scaleb = const.tile([128, K], F32)
        nc.sync.dma_start(scaleb[:], scaleb_d)
        biasb = const.tile([128, K], F32)
        nc.sync.dma_start(biasb[:], biasb_d)
        cwneg = const.tile([K, C], F32)
        nc.sync.dma_start(cwneg[:], cwneg_d)
        ident = const.tile([128, 128], BF)
        nc.sync.dma_start(ident[:], ident_d)
        onescol = const.tile([128, 1], BF)
        nc.sync.dma_start(onescol[:], onescol_d)

        loop_cm = tc.For_i(0, reps, 1) if reps > 1 else None
        if loop_cm is not None:
            ctx.enter_context(loop_cm)

        for b in range(BPC):
            encB = psE.tile([K, C], F32)       # (k, c) accumulated over N
            awsumP = psA.tile([K, 1], F32)

            x2sb = x2pool.tile([128, NTILES], F32)
            nc.sync.dma_start(x2sb[:], x2_d[b])

            pend = []                          # (xT, aw) awaiting mm2

            def issue_mm2(ent, first, last):
                xT_, aw_ = ent
                nc.tensor.matmul(encB[:], aw_[:], xT_[:],
                                 start=first, stop=last,
                                 skip_group_check=True)
                nc.tensor.matmul(awsumP[:], aw_[:], onescol[:],
                                 start=first, stop=last,
                                 skip_group_check=True)

            for ch in range(NCH):
                xh_t = xpool.tile([128, CCH, NC], BF)
                nc.sync.dma_start(
                    xh_t[:],
                    xh_d[b, :, :, ch * NC:(ch + 1) * NC].rearrange("c p n -> p c n"))

                for ti in range(NT):
                    gi = ch * NT + ti
                    T = psT.tile([128, K], F32, tag="T")
                    Xp = psX.tile([128, C], BF)

                    for ci in range(CCH):
                        a = xh_t[:, ci, bass.ts(ti, 128)]
                        # same stationary operand for both -> weight reuse
                        nc.tensor.matmul(T[:], a, w1t[:, ci, :],
                                         start=(ci == 0), stop=(ci == CCH - 1))
                        nc.tensor.transpose(Xp[:, bass.ts(ci, 128)], a, ident[:])

                    xT = sbX.tile([128, C], BF)
                    if gi % 3 == 0:
                        nc.vector.tensor_copy(xT[:], Xp[:])
                    else:
                        nc.scalar.copy(xT[:], Xp[:])

                    z0 = sbZ.tile([128, K], F32, tag="z0")
                    nc.vector.scalar_tensor_tensor(
                        z0[:], scaleb[:], x2sb[:, gi:gi + 1], T[:],
                        op0=_mult, op1=_add)
                    z = sbZ.tile([128, K], F32, tag="z")
                    nc.vector.tensor_add(z[:], z0[:], biasb[:])

                    e = sbAw.tile([128, K], BF, tag="e")
                    d = sbSmall.tile([128, 1], F32, tag="d")
                    nc.scalar.activation(e[:], z[:],
                                         mybir.ActivationFunctionType.Exp,
                                         accum_out=d[:])
                    dinv = sbSmall.tile([128, 1], F32, tag="dinv")
                    nc.vector.reciprocal(dinv[:], d[:])
                    aw = sbAw.tile([128, K], BF, tag="aw")
                    nc.vector.tensor_scalar_mul(aw[:], e[:], dinv[:])

                    pend.append((xT, aw))
                    if len(pend) > PIPE:
                        ent = pend.pop(0)
                        issue_mm2(ent, gi - PIPE == 0, False)

            for j, ent in enumerate(pend):
                gi = NTILES - len(pend) + j
                issue_mm2(ent, gi == 0, gi == NTILES - 1)

            # batch tail: enc = encB + awsum * (-c)
            awsum_sb = sbSmall.tile([K, 1], F32, tag="awsum")
            nc.scalar.copy(awsum_sb[:], awsumP[:])
            encOut = sbOut.tile([K, C], F32, tag="encOut")
            nc.vector.scalar_tensor_tensor(
                encOut[:], cwneg[:], awsum_sb[:], encB[:],
                op0=_mult, op1=_add)
            nc.sync.dma_start(out_d[b], encOut[:])

    nc.finalize()
    return nc


def _prep_inputs(x, codewords, scale):
    xf = np.ascontiguousarray(x.reshape(B, C, N))
    xh = xf.astype(bf16)
    xh4 = xh.reshape(B, CCH, 128, N)

    cw64 = codewords.astype(np.float64)
    sc64 = scale.astype(np.float64)
    alpha = float(sc64.max())
    # Constant softmax offset m ~ alpha * x2: exact softmax is invariant to
    # any per-pixel-constant offset; it only has to keep exp() in range.
    x2flat = np.einsum('bcn,bcn->bn', xf.astype(np.float64), xf.astype(np.float64))
    x2lo, x2hi = float(x2flat.min()), float(x2flat.max())
    m = alpha * 0.5 * (x2lo + x2hi)
    spread = abs(alpha) * 0.5 * (x2hi - x2lo) + 10.0
    assert spread < 60.0, (
        f"constant-offset softmax unsafe: |max_k s - m| can reach {spread:.1f}"
    )
    c2 = (cw64 ** 2).sum(1)
    bias = (sc64 * c2 - m).astype(np.float32)
    w1 = (-2.0 * sc64[:, None] * cw64).astype(bf16)        # (K, C)
    w1t = np.ascontiguousarray(
        w1.T.reshape(CCH, 128, K).transpose(1, 0, 2))       # (128, CCH, K)
    scaleb = np.broadcast_to(scale.astype(np.float32), (128, K)).copy()
    biasb = np.broadcast_to(bias, (128, K)).copy()
    cwneg = np.ascontiguousarray(-codewords.astype(np.float32))  # (K, C)

    # x2 tiled as (B, 128, NTILES): x2p[b, p, g] = x2[b, 128 g + p]
    x2p = np.ascontiguousarray(
        x2flat.astype(np.float32).reshape(B, NTILES, 128).transpose(0, 2, 1))

    consts = {
        "w1t": w1t,
        "biasb": biasb,
        "scaleb": scaleb,
        "cwneg": cwneg,
        "ident": np.eye(128, dtype=bf16),
        "ones_col": np.ones((128, 1), bf16),
    }
    in_maps = []
    for core in range(NCORES):
        m_ = dict(consts)
        m_["xh"] = xh4[core * BPC:(core + 1) * BPC]
        m_["x2p"] = x2p[core * BPC:(core + 1) * BPC]
        in_maps.append(m_)
    return in_maps


def kernel(x, codewords, scale, _trace=False, _return_results=False, _reps=1):
    key = ("prog", _reps)
    if key not in _compiled:
        _compiled[key] = _build_program(reps=_reps)
    nc = _compiled[key]
    in_maps = _prep_inputs(np.asarray(x), np.asarray(codewords),
                           np.asarray(scale))
    res = run_bass_kernel_spmd(nc, in_maps, list(range(NCORES)), trace=_trace)
    out = np.empty((B, K, C), np.float32)
    for core in range(NCORES):
        o = res.results[core]["enc"]                        # (BPC, K, C)
        for b in range(BPC):
            out[core * BPC + b] = o[b]
    if _return_results:
        return out, res
    return out
